# revision 1
# baseline (speedup 1.0000x reference)
"""Trainium2 Bass kernel for causal self-attention (B=2, S=2048, D=1024, H=16).

Sharding: 8 cores = 2 (batch) x 4 (head groups of 4 heads) — data parallel on
batch, tensor parallel on heads. Each core computes, for its batch b and its
4 heads (256 of the 1024 model dims):

  qT/kT = Wq_slice^T x^T            transposed layouts [head_dim, seq], fp16
  v     = x Wv_slice                natural layout [seq, head_dim], fp16
  per head pair (2 heads share the 128 partitions):
    scoresT[kv, q] blocks on PE (two row-packed K=64 matmuls),
    exp on ACT (psum -> fp16 sbuf), causal mask multiply on DVE (fp16 2x),
    P^T V + replicated ones-row denominators on PE (col-packed M=64),
    normalize: reciprocal_approx_fast + one tensor_mul.
  oT_partial = Wo_slice^T attnT     [1024, seq] fp32 partial

Host: feeds x^T and fp16 weight slices, sums the 4 partials per batch
(the "all-reduce" of the o-projection), transposes, adds bo.

All matmuls run in fp16 (1 cyc/row on PE) with fp32 PSUM accumulation;
softmax scale 1/sqrt(64) is folded into Wq on the host. Projections are
interleaved with the (ACT-bound) attention loop in program order, input DMAs
are split per contraction tile so compute starts early, and diagonal blocks
are column-sliced to skip fully-masked work.
"""

import numpy as np

import concourse.bacc as bacc
import concourse.tile as tile
from concourse import mybir
from concourse.bass_utils import run_bass_kernel_spmd

B, S, D, H = 2, 2048, 1024, 16
HD = D // H          # 64
P = 128
NCORES = 8
GROUPS = 4           # head groups (tensor parallel)
HPG = H // GROUPS    # 4 heads per group
CD = HPG * HD        # 256 local head dims per core
QT = 512             # q tile (matmul free dim)
KT = 128             # kv tile (psum partition dim)
NQT = S // QT        # 4
NKT = S // KT        # 16
KD = D // P          # 8 contraction tiles over the model dim

F32 = mybir.dt.float32
F16 = mybir.dt.float16
EXP = mybir.ActivationFunctionType.Exp

_NC_CACHE = {}


def _build_nc():
    if "nc" in _NC_CACHE:
        return _NC_CACHE["nc"]
    nc = bacc.Bacc()
    xt = nc.declare_dram_parameter("xt", [D, S], F16, isOutput=False)
    wq = nc.declare_dram_parameter("wq", [D, CD], F16, isOutput=False)
    wk = nc.declare_dram_parameter("wk", [D, CD], F16, isOutput=False)
    wv = nc.declare_dram_parameter("wv", [D, CD], F16, isOutput=False)
    wo = nc.declare_dram_parameter("wo", [CD, D], F16, isOutput=False)
    bq = nc.declare_dram_parameter("bq", [CD], F32, isOutput=False)
    bk = nc.declare_dram_parameter("bk", [CD], F32, isOutput=False)
    bv = nc.declare_dram_parameter("bv", [HPG, HD], F32, isOutput=False)
    msk = nc.declare_dram_parameter("msk", [4, P, 2 * QT], F16, isOutput=False)
    ot = nc.declare_dram_parameter("ot", [D, S], F32, isOutput=True)

    import concourse.bass as bass

    with tile.TileContext(nc) as tc:
        with tc.tile_pool(name="consts", bufs=1) as consts, \
             tc.tile_pool(name="work", bufs=3) as work, \
             tc.tile_pool(name="ps_s", bufs=2, space="PSUM") as ps_s, \
             tc.tile_pool(name="ps_av", bufs=2, space="PSUM") as ps_av, \
             tc.tile_pool(name="ps_po", bufs=2, space="PSUM") as ps_po:

            # ---- constant / persistent SBUF tensors ----
            xt_sb = consts.tile([P, KD, S], F16)
            wq_sb = consts.tile([P, KD, CD], F16)
            wk_sb = consts.tile([P, KD, CD], F16)
            wv_sb = consts.tile([P, KD, CD], F16)
            wo_sb = consts.tile([P, 2, D], F16)
            bq_sb = consts.tile([P, 2], F32)
            bk_sb = consts.tile([P, 2], F32)
            bv_sb = consts.tile([P, HPG, HD], F32)
            msk_sb = consts.tile([P, 4, 2 * QT], F16)
            qT_sb = consts.tile([P, 2, S], F16)
            kT_sb = consts.tile([P, 2, S], F16)
            v2_sb = consts.tile([P, NKT, 2, 3 * HD], F16)
            aT_sb = consts.tile([P, 2, NQT, QT], F16)

            # ---- input DMAs, split per contraction tile so compute can
            # start as soon as the first chunks land ----
            nc.sync.dma_start(out=bq_sb, in_=bq[:].rearrange("(m p) -> p m", p=P))
            nc.sync.dma_start(out=bk_sb, in_=bk[:].rearrange("(m p) -> p m", p=P))
            bv_ap = bv[:, :]
            bv_bc = bass.AP(tensor=bv_ap.tensor, offset=bv_ap.offset,
                            ap=[[0, P]] + list(bv_ap.ap))
            nc.gpsimd.dma_start(out=bv_sb, in_=bv_bc)
            nc.vector.memset(v2_sb[:, :, :, HD:2 * HD], 1.0)
            wq_r = wq[:, :].rearrange("(k p) c -> p k c", p=P)
            wk_r = wk[:, :].rearrange("(k p) c -> p k c", p=P)
            wv_r = wv[:, :].rearrange("(k p) c -> p k c", p=P)
            xt_r = xt[:, :].rearrange("(k p) s -> p k s", p=P)
            for kt in range(KD):
                nc.sync.dma_start(out=wq_sb[:, kt, :], in_=wq_r[:, kt, :])
                nc.sync.dma_start(out=wk_sb[:, kt, :], in_=wk_r[:, kt, :])
                nc.sync.dma_start(out=wv_sb[:, kt, :], in_=wv_r[:, kt, :])
                nc.sync.dma_start(out=xt_sb[:, kt, :], in_=xt_r[:, kt, :])
                if kt == 0:
                    nc.sync.dma_start(
                        out=msk_sb, in_=msk[:, :, :].rearrange("r p c -> p r c"))
            nc.sync.dma_start(out=wo_sb,
                              in_=wo[:, :].rearrange("(g p) e -> p g e", p=P))

            # ---- helpers ----
            def proj_qk(w_sb, b_sb, dst, mt, nts):
                pss = [ps_po.tile([P, QT], F32, tag="po", name=f"ps_qk{j}")
                       for j in range(len(nts))]
                for kt in range(KD):
                    lhs = w_sb[:, kt, mt * P:(mt + 1) * P]
                    for j, nt in enumerate(nts):
                        nc.tensor.matmul(
                            pss[j], lhs,
                            xt_sb[:, kt, nt * QT:(nt + 1) * QT],
                            start=(kt == 0), stop=(kt == KD - 1))
                for j, nt in enumerate(nts):
                    nc.vector.tensor_scalar_add(
                        dst[:, mt, nt * QT:(nt + 1) * QT], pss[j],
                        b_sb[:, mt:mt + 1])

            def proj_v(jt0, jt1):
                for jt in range(jt0, jt1):
                    ps = ps_po.tile([P, QT], F32, tag="po", name="ps_v")
                    for kt in range(KD):
                        nc.tensor.matmul(
                            ps[:, :CD], xt_sb[:, kt, jt * P:(jt + 1) * P],
                            wv_sb[:, kt, :],
                            start=(kt == 0), stop=(kt == KD - 1))
                    psh = ps[:, :CD].rearrange("p (h d) -> p h d", h=HPG)
                    # even heads -> cols 0:64, odd heads -> cols 128:192
                    nc.vector.tensor_add(
                        v2_sb[:, jt, :, 0:HD], psh[:, 0::2, :], bv_sb[:, 0::2, :])
                    nc.vector.tensor_add(
                        v2_sb[:, jt, :, 2 * HD:3 * HD], psh[:, 1::2, :],
                        bv_sb[:, 1::2, :])

            def attention(t, g, fillers=()):
                n_kv = 4 * (t + 1)
                # bank A: rows 0:64 = attn h(2g), rows 64:128 = denom h(2g)
                # bank B: rows 0:64 = denom h(2g+1), rows 64:128 = attn h(2g+1)
                av_a = ps_av.tile([P, QT], F32, tag="avden", name="av_a")
                av_b = ps_av.tile([P, QT], F32, tag="avden", name="av_b")
                for kv in range(n_kv):
                    if kv < len(fillers) and fillers[kv] is not None:
                        fillers[kv]()
                    r = kv - 4 * t
                    v0 = KT * r if r >= 1 else 0    # first valid q col
                    s = ps_s.tile([P, 2 * QT], F32, tag="s", name="s")
                    for idx in range(2):
                        p0 = 64 * idx
                        nc.tensor.matmul(
                            s[:, idx * QT + v0:(idx + 1) * QT],
                            kT_sb[p0:p0 + 64, g, kv * KT:(kv + 1) * KT],
                            qT_sb[p0:p0 + 64, g, t * QT + v0:(t + 1) * QT],
                            start=True, stop=True)
                    p_t = work.tile([P, 2 * QT], F16, tag="pt", name="p_t")
                    if r < 1:
                        nc.scalar.activation(p_t, s, EXP)
                    else:
                        for idx in range(2):
                            sl = slice(idx * QT + v0, (idx + 1) * QT)
                            nc.scalar.activation(p_t[:, sl], s[:, sl], EXP)
                    if r >= 0:
                        for idx in range(2):
                            sl = slice(idx * QT + v0, (idx + 1) * QT)
                            nc.vector.tensor_mul(p_t[:, sl], p_t[:, sl],
                                                 msk_sb[:, r, sl])
                    for idx, bank in ((0, av_a), (1, av_b)):
                        rhs = p_t[:, idx * QT + v0:(idx + 1) * QT]
                        lhsT = v2_sb[:, kv, g, HD * idx:HD * idx + 2 * HD]
                        nc.tensor.matmul(
                            bank[:, v0:], lhsT, rhs,
                            start=(kv == 0), stop=(kv == n_kv - 1))
                # normalize: aT = av / den, denominators shifted across
                # partition halves via a small SBUF->SBUF DMA
                rca = work.tile([P, QT], F32, tag="rca", name="rca")
                rcb = work.tile([P, QT], F32, tag="rcb", name="rcb")
                rc2 = work.tile([P, QT], F32, tag="rc2", name="rc2")
                nc.vector.reciprocal_approx_fast(rca, av_a)
                nc.vector.reciprocal_approx_fast(rcb, av_b)
                nc.sync.dma_start(out=rc2[0:64, :], in_=rca[64:128, :])
                nc.sync.dma_start(out=rc2[64:128, :], in_=rcb[0:64, :])
                nc.vector.tensor_mul(aT_sb[0:64, g, t, :], av_a[0:64, :],
                                     rc2[0:64, :])
                nc.vector.tensor_mul(aT_sb[64:128, g, t, :], av_b[64:128, :],
                                     rc2[64:128, :])

            def oproj(t, mts=None):
                for mt_e in (range(D // P) if mts is None else mts):
                    ps = ps_po.tile([P, QT], F32, tag="po", name="ps_o")
                    for g in range(2):
                        nc.tensor.matmul(
                            ps, wo_sb[:, g, mt_e * P:(mt_e + 1) * P],
                            aT_sb[:, g, t, :],
                            start=(g == 0), stop=(g == 1))
                    ot_t = work.tile([P, QT], F32, tag="ot", name="ot_t")
                    nc.vector.tensor_copy(ot_t, ps)
                    nc.sync.dma_start(
                        out=ot[mt_e * P:(mt_e + 1) * P, t * QT:(t + 1) * QT],
                        in_=ot_t)

            # ---- filler-interleaved schedule: the PE stream alternates
            # one projection/o-proj chunk per attention kv-iteration so the
            # in-order PE queue never sits on a block of non-attention work
            # while ACT starves ----
            def fq(mt, nt):
                return lambda: proj_qk(wq_sb, bq_sb, qT_sb, mt, [nt])

            def fk(mt, nt):
                return lambda: proj_qk(wk_sb, bk_sb, kT_sb, mt, [nt])

            def fv(jt):
                return lambda: proj_v(jt, jt + 1)

            def fo(t, m0):
                return lambda: oproj(t, mts=[m0, m0 + 1])

            # prefix: just enough for att(0,0)
            proj_qk(wq_sb, bq_sb, qT_sb, 0, [0])
            proj_qk(wk_sb, bk_sb, kT_sb, 0, [0])
            proj_v(0, 1)
            attention(0, 0, [fv(1), fv(2), fv(3), fq(1, 0)])
            attention(0, 1, [fk(1, 0), fq(0, 1), fk(0, 1), fq(1, 1)])
            attention(1, 0, [fk(1, 1), fo(0, 0), fo(0, 2), fo(0, 4),
                             fv(4), fv(5), fv(6), fv(7)])
            attention(1, 1, [fo(0, 6), fq(0, 2), fk(0, 2), fq(1, 2),
                             fk(1, 2)])
            attention(2, 0, [fo(1, 0), fo(1, 2), fo(1, 4), fo(1, 6),
                             None, None, None, None,
                             fv(8), fv(9), fv(10), fv(11)])
            attention(2, 1, [fq(0, 3), fk(0, 3), fq(1, 3), fk(1, 3)])
            attention(3, 0, [fo(2, 0), fo(2, 2), fo(2, 4), fo(2, 6),
                             None, None, None, None, None, None, None, None,
                             fv(12), fv(13), fv(14), fv(15)])
            attention(3, 1)
            oproj(3)

    nc.compile()
    _NC_CACHE["nc"] = nc
    return nc


def _make_masks():
    # msk[r, p, c] for the 4 diagonal kv offsets r: valid iff p <= (c % 512) - 128 r
    m = np.zeros((4, P, 2 * QT), dtype=np.float16)
    pp = np.arange(P)[:, None]
    cc = np.arange(QT)[None, :]
    for r in range(4):
        half = (pp <= cc - KT * r).astype(np.float16)
        m[r, :, :QT] = half
        m[r, :, QT:] = half
    return m


def _in_maps(x, Wq, bq, Wk, bk, Wv, bv, Wo):
    scale = np.float32(1.0 / np.sqrt(HD))
    masks = _make_masks()
    maps = []
    for core in range(NCORES):
        b, g = divmod(core, GROUPS)
        csl = slice(g * CD, (g + 1) * CD)
        maps.append({
            "xt": np.ascontiguousarray(x[b].T).astype(np.float16),
            "wq": np.ascontiguousarray(Wq[:, csl] * scale).astype(np.float16),
            "wk": np.ascontiguousarray(Wk[:, csl]).astype(np.float16),
            "wv": np.ascontiguousarray(Wv[:, csl]).astype(np.float16),
            "wo": np.ascontiguousarray(Wo[csl, :]).astype(np.float16),
            "bq": np.ascontiguousarray(bq[csl] * scale).astype(np.float32),
            "bk": np.ascontiguousarray(bk[csl]).astype(np.float32),
            "bv": np.ascontiguousarray(bv[csl]).reshape(HPG, HD).astype(np.float32),
            "msk": masks,
        })
    return maps


def kernel_with_results(x, Wq, bq, Wk, bk, Wv, bv, Wo, bo, trace=False):
    nc = _build_nc()
    maps = _in_maps(x, Wq, bq, Wk, bk, Wv, bv, Wo)
    kwargs = {}
    if trace:
        kwargs = dict(trace=True, trace_cores=[0])
    res = run_bass_kernel_spmd(nc, maps, core_ids=list(range(NCORES)), **kwargs)
    out = np.zeros((B, S, D), dtype=np.float32)
    for b in range(B):
        acc = np.zeros((D, S), dtype=np.float32)
        for g in range(GROUPS):
            acc += res.results[b * GROUPS + g]["ot"]
        out[b] = acc.T + np.asarray(bo, dtype=np.float32)[None, :]
    return out, res


def kernel(x, Wq, bq, Wk, bk, Wv, bv, Wo, bo):
    out, _ = kernel_with_results(x, Wq, bq, Wk, bk, Wv, bv, Wo, bo, trace=False)
    return out



# revision 4
# speedup vs baseline: 1.0135x; 1.0135x over previous
"""Trainium2 Bass kernel for causal self-attention (B=2, S=2048, D=1024, H=16).

Sharding: 8 cores = 2 (batch) x 4 (head groups of 4 heads) — data parallel on
batch, tensor parallel on heads. Each core computes, for its batch b and its
4 heads (256 of the 1024 model dims):

  qT/kT = Wq_slice^T x^T            transposed layouts [head_dim, seq], fp16
  v     = x Wv_slice                natural layout [seq, head_dim], fp16
  per head pair (2 heads share the 128 partitions):
    scoresT[kv, q] on PE (two row-packed K=64 matmuls into separate banks),
    per-head exp on ACT (psum -> fp16 sbuf), causal strip-mask mul on DVE,
    P^T V + replicated ones-row denominators on PE (col-packed M=64),
    normalize: reciprocal_approx_fast + one tensor_mul per head.
  oT_partial = Wo_slice^T attnT     [1024, seq] fp32 partial

Host: feeds x^T and fp16 weight slices, sums the 4 partials per batch
(the "all-reduce" of the o-projection), transposes, adds bo.

Scheduling: the PE instruction stream is software-pipelined — PV(kv-1) is
emitted after scores(kv) so the in-order PE queue never head-of-line blocks
on the ACT exp latency. All projection / o-projection work is decomposed
into ~216-432ns micro-tasks consumed from an ordered queue by small
per-block "pull" budgets (plus ensure() barriers for correctness), keeping
the PE continuously busy: TRN2 DVFS only reaches the full 2.4 GHz PE clock
after ~3us of uninterrupted execution, so every gap costs double. Input
DMAs are split per (contraction-tile, q-quarter) so the first projections
start as soon as the first chunks land; the causal mask is a single shared
[128, 512] strip; o-proj PSUM->SBUF copies run on the idle GpSimd engine.
"""

import numpy as np

import concourse.bacc as bacc
import concourse.tile as tile
from concourse import mybir
from concourse.bass_utils import run_bass_kernel_spmd

B, S, D, H = 2, 2048, 1024, 16
HD = D // H          # 64
P = 128
NCORES = 8
GROUPS = 4           # head groups (tensor parallel)
HPG = H // GROUPS    # 4 heads per group
CD = HPG * HD        # 256 local head dims per core
QT = 512             # q tile (matmul free dim)
KT = 128             # kv tile (psum partition dim)
NQT = S // QT        # 4
NKT = S // KT        # 16
KD = D // P          # 8 contraction tiles over the model dim

F32 = mybir.dt.float32
F16 = mybir.dt.float16
EXP = mybir.ActivationFunctionType.Exp

_NC_CACHE = {}


class TaskQ:
    """Ordered queue of PE micro-tasks with ns cost accounting."""

    def __init__(self):
        self.items = []          # (cost_ns, fn, completes_label)
        self.pos = 0
        self.done = set()

    def add(self, cost, fn, completes=None):
        self.items.append((cost, fn, completes))

    def _step(self):
        cost, fn, lab = self.items[self.pos]
        self.pos += 1
        fn()
        if lab:
            self.done.add(lab)
        return cost

    def pull(self, budget):
        while budget > 0 and self.pos < len(self.items):
            budget -= self._step()

    def ensure(self, label):
        while label not in self.done and self.pos < len(self.items):
            self._step()

    def drain(self):
        while self.pos < len(self.items):
            self._step()


def _build_nc():
    if "nc" in _NC_CACHE:
        return _NC_CACHE["nc"]
    nc = bacc.Bacc()
    xt = nc.declare_dram_parameter("xt", [D, S], F16, isOutput=False)
    wq = nc.declare_dram_parameter("wq", [D, CD], F16, isOutput=False)
    wk = nc.declare_dram_parameter("wk", [D, CD], F16, isOutput=False)
    wv = nc.declare_dram_parameter("wv", [D, CD], F16, isOutput=False)
    wo = nc.declare_dram_parameter("wo", [CD, D], F16, isOutput=False)
    bq = nc.declare_dram_parameter("bq", [CD], F32, isOutput=False)
    bk = nc.declare_dram_parameter("bk", [CD], F32, isOutput=False)
    bv = nc.declare_dram_parameter("bv", [HPG, HD], F32, isOutput=False)
    msk = nc.declare_dram_parameter("msk", [P, QT], F16, isOutput=False)
    ot = nc.declare_dram_parameter("ot", [D, S], F32, isOutput=True)

    import concourse.bass as bass

    with tile.TileContext(nc) as tc:
        with tc.tile_pool(name="consts", bufs=1) as consts, \
             tc.tile_pool(name="pwork", bufs=4) as pwork, \
             tc.tile_pool(name="misc", bufs=6) as misc, \
             tc.tile_pool(name="ps_s", bufs=4, space="PSUM") as ps_s, \
             tc.tile_pool(name="ps_av", bufs=2, space="PSUM") as ps_av, \
             tc.tile_pool(name="ps_po", bufs=2, space="PSUM") as ps_po:

            # ---- constant / persistent SBUF tensors ----
            xt_sb = consts.tile([P, KD, S], F16)
            wq_sb = consts.tile([P, KD, CD], F16)
            wk_sb = consts.tile([P, KD, CD], F16)
            wv_sb = consts.tile([P, KD, CD], F16)
            wo_sb = consts.tile([P, 2, D], F16)
            bq_sb = consts.tile([P, 2], F32)
            bk_sb = consts.tile([P, 2], F32)
            bv_sb = consts.tile([P, HPG, HD], F32)
            msk_sb = consts.tile([P, QT], F16)
            qT_sb = consts.tile([P, 2, S], F16)
            kT_sb = consts.tile([P, 2, S], F16)
            v2_sb = consts.tile([P, NKT, 2, 3 * HD], F16)
            aT_sb = consts.tile([P, 2, NQT, QT], F16)

            # ---- input DMAs: small consts, then per-(kt, q-quarter) chunks
            # ordered so the prefix projections (q columns 0:512) can start
            # as soon as the first contraction tiles land ----
            nc.sync.dma_start(out=bq_sb, in_=bq[:].rearrange("(m p) -> p m", p=P))
            nc.sync.dma_start(out=bk_sb, in_=bk[:].rearrange("(m p) -> p m", p=P))
            bv_ap = bv[:, :]
            bv_bc = bass.AP(tensor=bv_ap.tensor, offset=bv_ap.offset,
                            ap=[[0, P]] + list(bv_ap.ap))
            nc.gpsimd.dma_start(out=bv_sb, in_=bv_bc)
            nc.sync.dma_start(out=msk_sb, in_=msk[:, :])
            nc.vector.memset(v2_sb[:, :, :, HD:2 * HD], 1.0)
            wq_r = wq[:, :].rearrange("(k p) c -> p k c", p=P)
            wk_r = wk[:, :].rearrange("(k p) c -> p k c", p=P)
            wv_r = wv[:, :].rearrange("(k p) c -> p k c", p=P)
            xt_r = xt[:, :].rearrange("(k p) s -> p k s", p=P)
            for kt in range(KD):
                nc.sync.dma_start(out=wq_sb[:, kt, :], in_=wq_r[:, kt, :])
                nc.sync.dma_start(out=wk_sb[:, kt, :], in_=wk_r[:, kt, :])
                nc.sync.dma_start(out=xt_sb[:, kt, 0:QT], in_=xt_r[:, kt, 0:QT])
                nc.sync.dma_start(out=wv_sb[:, kt, :], in_=wv_r[:, kt, :])
            for nt in range(1, NQT):
                sl = slice(nt * QT, (nt + 1) * QT)
                for kt in range(KD):
                    nc.sync.dma_start(out=xt_sb[:, kt, sl], in_=xt_r[:, kt, sl])
            nc.sync.dma_start(out=wo_sb,
                              in_=wo[:, :].rearrange("(g p) e -> p g e", p=P))

            # ---- micro-task generators ----
            q = TaskQ()

            def add_gqk(w_sb, b_sb, dst, g, nt, lab):
                cell = {}

                def mk(kt):
                    def f():
                        if kt == 0:
                            cell["ps"] = ps_po.tile([P, QT], F32, tag="po",
                                                    name=f"ps_{lab}")
                        nc.tensor.matmul(
                            cell["ps"], w_sb[:, kt, g * P:(g + 1) * P],
                            xt_sb[:, kt, nt * QT:(nt + 1) * QT],
                            start=(kt == 0), stop=(kt == KD - 1))
                        if kt == KD - 1:
                            nc.vector.tensor_scalar_add(
                                dst[:, g, nt * QT:(nt + 1) * QT], cell["ps"],
                                b_sb[:, g:g + 1])
                    return f

                for kt in range(KD):
                    q.add(216, mk(kt), completes=(lab if kt == KD - 1 else None))

            def add_gv(jt):
                cell = {}

                def mk(k0):
                    def f():
                        if k0 == 0:
                            cell["ps"] = ps_po.tile([P, QT], F32, tag="po",
                                                    name=f"ps_v{jt}")
                        ps = cell["ps"]
                        for kt in (k0, k0 + 1):
                            nc.tensor.matmul(
                                ps[:, :CD], xt_sb[:, kt, jt * P:(jt + 1) * P],
                                wv_sb[:, kt, :],
                                start=(kt == 0), stop=(kt == KD - 1))
                        if k0 == KD - 2:
                            psh = ps[:, :CD].rearrange("p (h d) -> p h d", h=HPG)
                            nc.vector.tensor_add(
                                v2_sb[:, jt, :, 0:HD], psh[:, 0::2, :],
                                bv_sb[:, 0::2, :])
                            nc.vector.tensor_add(
                                v2_sb[:, jt, :, 2 * HD:3 * HD], psh[:, 1::2, :],
                                bv_sb[:, 1::2, :])
                    return f

                for k0 in range(0, KD, 2):
                    q.add(214, mk(k0),
                          completes=(f"v{jt}" if k0 == KD - 2 else None))

            def add_go(t):
                def mk(mt):
                    def f():
                        ps = ps_po.tile([P, QT], F32, tag="po", name=f"ps_o{t}")
                        for g in range(2):
                            nc.tensor.matmul(
                                ps, wo_sb[:, g, mt * P:(mt + 1) * P],
                                aT_sb[:, g, t, :],
                                start=(g == 0), stop=(g == 1))
                        ot_t = misc.tile([P, QT], F32, tag="m", name="ot_t")
                        nc.vector.tensor_copy(ot_t, ps)
                        nc.sync.dma_start(
                            out=ot[mt * P:(mt + 1) * P, t * QT:(t + 1) * QT],
                            in_=ot_t)
                    return f

                for mt in range(D // P):
                    q.add(432, mk(mt))

            # queue order = dependency-feasible emission order; ensure()
            # barriers in attention() guarantee correctness regardless of
            # the pull budgets.
            for jt in (1, 2, 3):
                add_gv(jt)
            add_gqk(wq_sb, bq_sb, qT_sb, 1, 0, "q10")
            add_gqk(wk_sb, bk_sb, kT_sb, 1, 0, "k10")
            add_gqk(wq_sb, bq_sb, qT_sb, 0, 1, "q01")
            add_gqk(wk_sb, bk_sb, kT_sb, 0, 1, "k01")
            add_gqk(wq_sb, bq_sb, qT_sb, 1, 1, "q11")
            add_gqk(wk_sb, bk_sb, kT_sb, 1, 1, "k11")
            for jt in (4, 5, 6, 7):
                add_gv(jt)
            add_go(0)
            add_gqk(wq_sb, bq_sb, qT_sb, 0, 2, "q02")
            add_gqk(wk_sb, bk_sb, kT_sb, 0, 2, "k02")
            add_gqk(wq_sb, bq_sb, qT_sb, 1, 2, "q12")
            add_gqk(wk_sb, bk_sb, kT_sb, 1, 2, "k12")
            for jt in (8, 9, 10, 11):
                add_gv(jt)
            add_go(1)
            add_gqk(wq_sb, bq_sb, qT_sb, 0, 3, "q03")
            add_gqk(wk_sb, bk_sb, kT_sb, 0, 3, "k03")
            for jt in (12, 13, 14, 15):
                add_gv(jt)
            add_gqk(wq_sb, bq_sb, qT_sb, 1, 3, "q13")
            add_gqk(wk_sb, bk_sb, kT_sb, 1, 3, "k13")
            add_go(2)

            # ---- prefix: q(0,0), k(0,0), v(0) interleaved per kt chunk so
            # compute starts as soon as the first input chunks land ----
            ps_q0 = ps_s.tile([P, QT], F32, tag="s", name="pfx_q")
            ps_k0 = ps_s.tile([P, QT], F32, tag="s", name="pfx_k")
            ps_v0 = ps_po.tile([P, QT], F32, tag="po", name="pfx_v")
            for kt in range(KD):
                nc.tensor.matmul(ps_q0, wq_sb[:, kt, 0:P], xt_sb[:, kt, 0:QT],
                                 start=(kt == 0), stop=(kt == KD - 1))
                nc.tensor.matmul(ps_k0, wk_sb[:, kt, 0:P], xt_sb[:, kt, 0:QT],
                                 start=(kt == 0), stop=(kt == KD - 1))
                nc.tensor.matmul(ps_v0[:, :CD], xt_sb[:, kt, 0:P],
                                 wv_sb[:, kt, :],
                                 start=(kt == 0), stop=(kt == KD - 1))
            nc.vector.tensor_scalar_add(qT_sb[:, 0, 0:QT], ps_q0, bq_sb[:, 0:1])
            nc.vector.tensor_scalar_add(kT_sb[:, 0, 0:QT], ps_k0, bk_sb[:, 0:1])
            psh0 = ps_v0[:, :CD].rearrange("p (h d) -> p h d", h=HPG)
            nc.vector.tensor_add(v2_sb[:, 0, :, 0:HD], psh0[:, 0::2, :],
                                 bv_sb[:, 0::2, :])
            nc.vector.tensor_add(v2_sb[:, 0, :, 2 * HD:3 * HD], psh0[:, 1::2, :],
                                 bv_sb[:, 1::2, :])
            for lab in ("q00", "k00", "v0"):
                q.done.add(lab)

            # ---- pipelined attention ----
            PRE_D = [600, 450, 350, 250]    # diag pulls by r
            MID_D = [350, 250, 200, 150]
            PRE_N, MID_N = 100, 200         # non-diag
            SEG_START, SEG_END = 1000, 500

            def flush_pv(pend, g, av_a, av_b, n_kv, mid):
                kv, pA, pB, v0 = pend
                st, sp = (kv == 0), (kv == n_kv - 1)
                q.ensure(f"v{kv}")
                nc.tensor.matmul(av_a[:, v0:], v2_sb[:, kv, g, 0:2 * HD],
                                 pA[:, v0:], start=st, stop=sp)
                q.pull(mid)
                nc.tensor.matmul(av_b[:, v0:], v2_sb[:, kv, g, HD:3 * HD],
                                 pB[:, v0:], start=st, stop=sp)

            def attention(t, g):
                n_kv = 4 * (t + 1)
                q.ensure(f"q{g}{t}")
                av_a = ps_av.tile([P, QT], F32, tag="av", name="av_a")
                av_b = ps_av.tile([P, QT], F32, tag="av", name="av_b")
                pend = None
                for kv in range(n_kv):
                    r = kv - 4 * t
                    v0 = KT * r if r >= 1 else 0
                    q.ensure(f"k{g}{kv // 4}")
                    if t > 0:
                        pull = (PRE_D[r] if r >= 0 else PRE_N)
                        if kv == 0:
                            pull += SEG_START
                        q.pull(pull)
                    sA = ps_s.tile([P, QT], F32, tag="s", name="sA")
                    sB = ps_s.tile([P, QT], F32, tag="s", name="sB")
                    kvs = slice(kv * KT, (kv + 1) * KT)
                    qs = slice(t * QT + v0, (t + 1) * QT)
                    nc.tensor.matmul(sA[:, v0:], kT_sb[0:64, g, kvs],
                                     qT_sb[0:64, g, qs], start=True, stop=True)
                    nc.tensor.matmul(sB[:, v0:], kT_sb[64:128, g, kvs],
                                     qT_sb[64:128, g, qs], start=True, stop=True)
                    pA = pwork.tile([P, QT], F16, tag="p", name="pA")
                    pB = pwork.tile([P, QT], F16, tag="p", name="pB")
                    nc.scalar.activation(pA[:, v0:], sA[:, v0:], EXP)
                    nc.scalar.activation(pB[:, v0:], sB[:, v0:], EXP)
                    if r >= 0:
                        nc.vector.tensor_mul(pA[:, v0:], pA[:, v0:],
                                             msk_sb[:, 0:QT - v0])
                        nc.vector.tensor_mul(pB[:, v0:], pB[:, v0:],
                                             msk_sb[:, 0:QT - v0])
                    if pend is not None:
                        pr = kv - 1 - 4 * t
                        mid = (MID_D[pr] if pr >= 0 else MID_N) if t > 0 else 0
                        flush_pv(pend, g, av_a, av_b, n_kv, mid)
                    pend = (kv, pA, pB, v0)
                if t > 0:
                    q.pull(SEG_END)
                flush_pv(pend, g, av_a, av_b, n_kv, 0)
                # normalize: aT = av / den, denominators shifted across
                # partition halves via a small SBUF->SBUF DMA
                rca = misc.tile([P, QT], F32, tag="m", name="rca")
                rcb = misc.tile([P, QT], F32, tag="m", name="rcb")
                rc2 = misc.tile([P, QT], F32, tag="m", name="rc2")
                nc.vector.reciprocal_approx_fast(rca, av_a)
                nc.vector.reciprocal_approx_fast(rcb, av_b)
                nc.sync.dma_start(out=rc2[0:64, :], in_=rca[64:128, :])
                nc.sync.dma_start(out=rc2[64:128, :], in_=rcb[0:64, :])
                nc.vector.tensor_mul(aT_sb[0:64, g, t, :], av_a[0:64, :],
                                     rc2[0:64, :])
                nc.vector.tensor_mul(aT_sb[64:128, g, t, :], av_b[64:128, :],
                                     rc2[64:128, :])

            for t in range(NQT):
                for g in range(2):
                    attention(t, g)
            q.drain()

            # ---- tail: o-projection of the last q tile ----
            for mt in range(D // P):
                ps = ps_po.tile([P, QT], F32, tag="po", name="ps_o3")
                for g in range(2):
                    nc.tensor.matmul(ps, wo_sb[:, g, mt * P:(mt + 1) * P],
                                     aT_sb[:, g, NQT - 1, :],
                                     start=(g == 0), stop=(g == 1))
                ot_t = misc.tile([P, QT], F32, tag="m", name="ot_t3")
                nc.vector.tensor_copy(ot_t, ps)
                nc.sync.dma_start(
                    out=ot[mt * P:(mt + 1) * P, (NQT - 1) * QT:NQT * QT],
                    in_=ot_t)

    nc.compile()
    _NC_CACHE["nc"] = nc
    return nc


def _make_strip():
    # strip[p, u] = 1.0 iff p <= u; diagonal kv offset r uses cols [0, 512-128r)
    pp = np.arange(P)[:, None]
    uu = np.arange(QT)[None, :]
    return (pp <= uu).astype(np.float16)


def _in_maps(x, Wq, bq, Wk, bk, Wv, bv, Wo):
    scale = np.float32(1.0 / np.sqrt(HD))
    strip = _make_strip()
    maps = []
    for core in range(NCORES):
        b, g = divmod(core, GROUPS)
        csl = slice(g * CD, (g + 1) * CD)
        maps.append({
            "xt": np.ascontiguousarray(x[b].T).astype(np.float16),
            "wq": np.ascontiguousarray(Wq[:, csl] * scale).astype(np.float16),
            "wk": np.ascontiguousarray(Wk[:, csl]).astype(np.float16),
            "wv": np.ascontiguousarray(Wv[:, csl]).astype(np.float16),
            "wo": np.ascontiguousarray(Wo[csl, :]).astype(np.float16),
            "bq": np.ascontiguousarray(bq[csl] * scale).astype(np.float32),
            "bk": np.ascontiguousarray(bk[csl]).astype(np.float32),
            "bv": np.ascontiguousarray(bv[csl]).reshape(HPG, HD).astype(np.float32),
            "msk": strip,
        })
    return maps


def kernel_with_results(x, Wq, bq, Wk, bk, Wv, bv, Wo, bo, trace=False):
    nc = _build_nc()
    maps = _in_maps(x, Wq, bq, Wk, bk, Wv, bv, Wo)
    kwargs = {}
    if trace:
        kwargs = dict(trace=True, trace_cores=[0])
    res = run_bass_kernel_spmd(nc, maps, core_ids=list(range(NCORES)), **kwargs)
    out = np.zeros((B, S, D), dtype=np.float32)
    for b in range(B):
        acc = np.zeros((D, S), dtype=np.float32)
        for g in range(GROUPS):
            acc += res.results[b * GROUPS + g]["ot"]
        out[b] = acc.T + np.asarray(bo, dtype=np.float32)[None, :]
    return out, res


def kernel(x, Wq, bq, Wk, bk, Wv, bv, Wo, bo):
    out, _ = kernel_with_results(x, Wq, bq, Wk, bk, Wv, bv, Wo, bo, trace=False)
    return out


# revision 5
# speedup vs baseline: 1.1086x; 1.0938x over previous
"""Trainium2 Bass kernel for causal self-attention (B=2, S=2048, D=1024, H=16).

Sharding: 8 cores = 2 (batch) x 4 (head groups of 4 heads) — data parallel on
batch, tensor parallel on heads. Each core computes, for its batch b and its
4 heads (256 of the 1024 model dims):

  qT/kT = Wq_slice^T x^T            transposed layouts [head_dim, seq], fp16
  v     = x Wv_slice                natural layout [seq, head_dim], fp16
  per head pair (2 heads share the 128 partitions):
    scoresT[kv, q] on PE (two row-packed K=64 matmuls into one 2-bank tile),
    one fused exp per block on ACT (strided AP covers both heads; psum ->
    fp16 sbuf), one causal strip-mask mul on DVE for diagonal blocks,
    P^T V + replicated ones-row denominators on PE (col-packed M=64),
    normalize: reciprocal_approx_fast + one tensor_mul per head.
  oT_partial = Wo_slice^T attnT     [1024, seq] fp32 partial

Host: feeds x^T and fp16 weight slices, sums the 4 partials per batch
(the "all-reduce" of the o-projection), transposes, adds bo.

Scheduling: the PE instruction stream is software-pipelined — PV(kv-1) is
emitted after scores(kv) so the in-order PE queue never head-of-line blocks
on the ACT exp latency. All projection / o-projection work is decomposed
into ~216-432ns micro-tasks consumed from an ordered queue by small
per-block "pull" budgets (plus ensure() barriers for correctness), keeping
the PE continuously busy: TRN2 DVFS only reaches the full 2.4 GHz PE clock
after ~3us of uninterrupted execution, so every gap costs double. A task
reserve keeps the last head-pair's q/k projections as fillers for the final
segment, and the o-projection of q-tile 2 is emitted between the last
softmax-normalize and the tail o-projection to cover that window. Input
DMAs are split per (contraction-tile, q-quarter) so the first projections
start as soon as the first chunks land; the causal mask is a shared
[128, 2, 512] strip.
"""

import numpy as np

import concourse.bacc as bacc
import concourse.tile as tile
from concourse import mybir
from concourse.bass_utils import run_bass_kernel_spmd

B, S, D, H = 2, 2048, 1024, 16
HD = D // H          # 64
P = 128
NCORES = 8
GROUPS = 4           # head groups (tensor parallel)
HPG = H // GROUPS    # 4 heads per group
CD = HPG * HD        # 256 local head dims per core
QT = 512             # q tile (matmul free dim)
KT = 128             # kv tile (psum partition dim)
NQT = S // QT        # 4
NKT = S // KT        # 16
KD = D // P          # 8 contraction tiles over the model dim

F32 = mybir.dt.float32
F16 = mybir.dt.float16
EXP = mybir.ActivationFunctionType.Exp

_NC_CACHE = {}


class TaskQ:
    """Ordered queue of PE micro-tasks with ns cost accounting."""

    def __init__(self):
        self.items = []          # (cost_ns, fn, completes_label)
        self.pos = 0
        self.done = set()
        self.reserve = 0
        self._remaining = 0

    def add(self, cost, fn, completes=None):
        self.items.append((cost, fn, completes))
        self._remaining += cost

    def _step(self):
        cost, fn, lab = self.items[self.pos]
        self.pos += 1
        self._remaining -= cost
        fn()
        if lab:
            self.done.add(lab)
        return cost

    def pull(self, budget):
        while (budget > 0 and self.pos < len(self.items)
               and self._remaining > self.reserve):
            budget -= self._step()

    def ensure(self, label):
        while label not in self.done and self.pos < len(self.items):
            self._step()

    def drain(self):
        while self.pos < len(self.items):
            self._step()


def _build_nc():
    if "nc" in _NC_CACHE:
        return _NC_CACHE["nc"]
    nc = bacc.Bacc()
    xt = nc.declare_dram_parameter("xt", [D, S], F16, isOutput=False)
    wq = nc.declare_dram_parameter("wq", [D, CD], F16, isOutput=False)
    wk = nc.declare_dram_parameter("wk", [D, CD], F16, isOutput=False)
    wv = nc.declare_dram_parameter("wv", [D, CD], F16, isOutput=False)
    wo = nc.declare_dram_parameter("wo", [CD, D], F16, isOutput=False)
    bq = nc.declare_dram_parameter("bq", [CD], F32, isOutput=False)
    bk = nc.declare_dram_parameter("bk", [CD], F32, isOutput=False)
    bv = nc.declare_dram_parameter("bv", [HPG, HD], F32, isOutput=False)
    msk = nc.declare_dram_parameter("msk", [P, QT], F16, isOutput=False)
    ot = nc.declare_dram_parameter("ot", [D, S], F32, isOutput=True)

    import concourse.bass as bass

    with tile.TileContext(nc) as tc:
        with tc.tile_pool(name="consts", bufs=1) as consts, \
             tc.tile_pool(name="pwork", bufs=3) as pwork, \
             tc.tile_pool(name="misc", bufs=6) as misc, \
             tc.tile_pool(name="ps_s", bufs=2, space="PSUM") as ps_s, \
             tc.tile_pool(name="ps_av", bufs=2, space="PSUM") as ps_av, \
             tc.tile_pool(name="ps_po", bufs=2, space="PSUM") as ps_po:

            # ---- constant / persistent SBUF tensors ----
            xt_sb = consts.tile([P, KD, S], F16)
            wq_sb = consts.tile([P, KD, CD], F16)
            wk_sb = consts.tile([P, KD, CD], F16)
            wv_sb = consts.tile([P, KD, CD], F16)
            wo_sb = consts.tile([P, 2, D], F16)
            bq_sb = consts.tile([P, 2], F32)
            bk_sb = consts.tile([P, 2], F32)
            bv_sb = consts.tile([P, HPG, HD], F32)
            msk_sb = consts.tile([P, 2, QT], F16)
            qT_sb = consts.tile([P, 2, S], F16)
            kT_sb = consts.tile([P, 2, S], F16)
            v2_sb = consts.tile([P, NKT, 2, 3 * HD], F16)
            aT_sb = consts.tile([P, 2, NQT, QT], F16)

            # ---- input DMAs: small consts, then per-(kt, q-quarter) chunks
            # ordered so the prefix q/k projections (q columns 0:512) can
            # start as soon as the first contraction tiles land ----
            nc.sync.dma_start(out=bq_sb, in_=bq[:].rearrange("(m p) -> p m", p=P))
            nc.sync.dma_start(out=bk_sb, in_=bk[:].rearrange("(m p) -> p m", p=P))
            bv_ap = bv[:, :]
            bv_bc = bass.AP(tensor=bv_ap.tensor, offset=bv_ap.offset,
                            ap=[[0, P]] + list(bv_ap.ap))
            nc.gpsimd.dma_start(out=bv_sb, in_=bv_bc)
            nc.vector.memset(v2_sb[:, :, :, HD:2 * HD], 1.0)
            wq_r = wq[:, :].rearrange("(k p) c -> p k c", p=P)
            wk_r = wk[:, :].rearrange("(k p) c -> p k c", p=P)
            wv_r = wv[:, :].rearrange("(k p) c -> p k c", p=P)
            xt_r = xt[:, :].rearrange("(k p) s -> p k s", p=P)
            for kt in range(KD):
                nc.sync.dma_start(out=wq_sb[:, kt, :], in_=wq_r[:, kt, :])
                nc.sync.dma_start(out=wk_sb[:, kt, :], in_=wk_r[:, kt, :])
                nc.sync.dma_start(out=xt_sb[:, kt, 0:QT], in_=xt_r[:, kt, 0:QT])
            nc.sync.dma_start(out=msk_sb[:, 0, :], in_=msk[:, :])
            nc.sync.dma_start(out=msk_sb[:, 1, :], in_=msk[:, :])
            for kt in range(KD):
                nc.sync.dma_start(out=wv_sb[:, kt, :], in_=wv_r[:, kt, :])
            for nt in range(1, NQT):
                sl = slice(nt * QT, (nt + 1) * QT)
                for kt in range(KD):
                    nc.sync.dma_start(out=xt_sb[:, kt, sl], in_=xt_r[:, kt, sl])
            nc.sync.dma_start(out=wo_sb,
                              in_=wo[:, :].rearrange("(g p) e -> p g e", p=P))

            # ---- micro-task generators ----
            q = TaskQ()

            def add_gqk(w_sb, b_sb, dst, g, nt, lab):
                cell = {}

                def mk(kt):
                    def f():
                        if kt == 0:
                            cell["ps"] = ps_po.tile([P, QT], F32, tag="po",
                                                    name=f"ps_{lab}")
                        nc.tensor.matmul(
                            cell["ps"], w_sb[:, kt, g * P:(g + 1) * P],
                            xt_sb[:, kt, nt * QT:(nt + 1) * QT],
                            start=(kt == 0), stop=(kt == KD - 1))
                        if kt == KD - 1:
                            nc.vector.tensor_scalar_add(
                                dst[:, g, nt * QT:(nt + 1) * QT], cell["ps"],
                                b_sb[:, g:g + 1])
                    return f

                for kt in range(KD):
                    q.add(216, mk(kt), completes=(lab if kt == KD - 1 else None))

            def add_gv(jt):
                cell = {}

                def mk(k0):
                    def f():
                        if k0 == 0:
                            cell["ps"] = ps_po.tile([P, QT], F32, tag="po",
                                                    name=f"ps_v{jt}")
                        ps = cell["ps"]
                        for kt in (k0, k0 + 1):
                            nc.tensor.matmul(
                                ps[:, :CD], xt_sb[:, kt, jt * P:(jt + 1) * P],
                                wv_sb[:, kt, :],
                                start=(kt == 0), stop=(kt == KD - 1))
                        if k0 == KD - 2:
                            psh = ps[:, :CD].rearrange("p (h d) -> p h d", h=HPG)
                            nc.vector.tensor_add(
                                v2_sb[:, jt, :, 0:HD], psh[:, 0::2, :],
                                bv_sb[:, 0::2, :])
                            nc.vector.tensor_add(
                                v2_sb[:, jt, :, 2 * HD:3 * HD], psh[:, 1::2, :],
                                bv_sb[:, 1::2, :])
                    return f

                for k0 in range(0, KD, 2):
                    q.add(214, mk(k0),
                          completes=(f"v{jt}" if k0 == KD - 2 else None))

            def emit_o(t, mt, copy_eng):
                ps = ps_po.tile([P, QT], F32, tag="po", name=f"ps_o{t}")
                for g in range(2):
                    nc.tensor.matmul(ps, wo_sb[:, g, mt * P:(mt + 1) * P],
                                     aT_sb[:, g, t, :],
                                     start=(g == 0), stop=(g == 1))
                ot_t = misc.tile([P, QT], F32, tag="m", name="ot_t")
                if copy_eng == "scalar":
                    nc.scalar.copy(ot_t, ps)
                else:
                    nc.vector.tensor_copy(ot_t, ps)
                nc.sync.dma_start(
                    out=ot[mt * P:(mt + 1) * P, t * QT:(t + 1) * QT],
                    in_=ot_t)

            def add_go(t):
                for mt in range(D // P):
                    q.add(432, lambda mt=mt: emit_o(t, mt, "vector"))

            # queue order = dependency-feasible emission order; ensure()
            # barriers in attention() guarantee correctness regardless of
            # the pull budgets.  go(2) is reserved for the tail window.
            for jt in (1, 2, 3):
                add_gv(jt)
            add_gqk(wq_sb, bq_sb, qT_sb, 1, 0, "q10")
            add_gqk(wk_sb, bk_sb, kT_sb, 1, 0, "k10")
            add_gqk(wq_sb, bq_sb, qT_sb, 0, 1, "q01")
            add_gqk(wk_sb, bk_sb, kT_sb, 0, 1, "k01")
            add_gqk(wq_sb, bq_sb, qT_sb, 1, 1, "q11")
            add_gqk(wk_sb, bk_sb, kT_sb, 1, 1, "k11")
            for jt in (4, 5, 6, 7):
                add_gv(jt)
            add_go(0)
            add_gqk(wq_sb, bq_sb, qT_sb, 0, 2, "q02")
            add_gqk(wk_sb, bk_sb, kT_sb, 0, 2, "k02")
            add_gqk(wq_sb, bq_sb, qT_sb, 1, 2, "q12")
            add_gqk(wk_sb, bk_sb, kT_sb, 1, 2, "k12")
            for jt in (8, 9, 10, 11):
                add_gv(jt)
            add_go(1)
            add_gqk(wq_sb, bq_sb, qT_sb, 0, 3, "q03")
            add_gqk(wk_sb, bk_sb, kT_sb, 0, 3, "k03")
            for jt in (12, 13, 14, 15):
                add_gv(jt)
            add_gqk(wq_sb, bq_sb, qT_sb, 1, 3, "q13")
            add_gqk(wk_sb, bk_sb, kT_sb, 1, 3, "k13")

            # ---- prefix: q(0,0) and k(0,0) interleaved per kt chunk (start
            # as soon as the first chunks land), then v(0) once wv arrives ----
            ps_q0 = ps_s.tile([P, 2 * QT], F32, tag="s", name="pfx_qk")
            for kt in range(KD):
                nc.tensor.matmul(ps_q0[:, 0:QT], wq_sb[:, kt, 0:P],
                                 xt_sb[:, kt, 0:QT],
                                 start=(kt == 0), stop=(kt == KD - 1))
                nc.tensor.matmul(ps_q0[:, QT:], wk_sb[:, kt, 0:P],
                                 xt_sb[:, kt, 0:QT],
                                 start=(kt == 0), stop=(kt == KD - 1))
            ps_v0 = ps_po.tile([P, QT], F32, tag="po", name="pfx_v")
            for kt in range(KD):
                nc.tensor.matmul(ps_v0[:, :CD], xt_sb[:, kt, 0:P],
                                 wv_sb[:, kt, :],
                                 start=(kt == 0), stop=(kt == KD - 1))
            nc.vector.tensor_scalar_add(qT_sb[:, 0, 0:QT], ps_q0[:, 0:QT],
                                        bq_sb[:, 0:1])
            nc.vector.tensor_scalar_add(kT_sb[:, 0, 0:QT], ps_q0[:, QT:],
                                        bk_sb[:, 0:1])
            psh0 = ps_v0[:, :CD].rearrange("p (h d) -> p h d", h=HPG)
            nc.vector.tensor_add(v2_sb[:, 0, :, 0:HD], psh0[:, 0::2, :],
                                 bv_sb[:, 0::2, :])
            nc.vector.tensor_add(v2_sb[:, 0, :, 2 * HD:3 * HD], psh0[:, 1::2, :],
                                 bv_sb[:, 1::2, :])
            for lab in ("q00", "k00", "v0"):
                q.done.add(lab)

            # ---- pipelined attention ----
            PRE_D = [650, 500, 400, 300]    # diag pulls by r
            MID_D = [400, 300, 250, 200]
            PRE_N, MID_N = 200, 250         # non-diag
            SEG_START, SEG_END = 1400, 600
            q.reserve = 5500                # keep fillers for att(3,1)

            def flush_pv(pend, g, av_a, av_b, n_kv, mid):
                kv, p2, v0 = pend
                st, sp = (kv == 0), (kv == n_kv - 1)
                q.ensure(f"v{kv}")
                nc.tensor.matmul(av_a[:, v0:], v2_sb[:, kv, g, 0:2 * HD],
                                 p2[:, v0:QT], start=st, stop=sp)
                q.pull(mid)
                nc.tensor.matmul(av_b[:, v0:], v2_sb[:, kv, g, HD:3 * HD],
                                 p2[:, QT + v0:], start=st, stop=sp)

            def attention(t, g):
                n_kv = 4 * (t + 1)
                q.ensure(f"q{g}{t}")
                av_a = ps_av.tile([P, QT], F32, tag="av", name="av_a")
                av_b = ps_av.tile([P, QT], F32, tag="av", name="av_b")
                pend = None
                for kv in range(n_kv):
                    r = kv - 4 * t
                    v0 = KT * r if r >= 1 else 0
                    q.ensure(f"k{g}{kv // 4}")
                    if t > 0:
                        pull = (PRE_D[r] if r >= 0 else PRE_N)
                        if kv == 0:
                            pull += SEG_START
                        q.pull(pull)
                    s2 = ps_s.tile([P, 2 * QT], F32, tag="s", name="s2")
                    kvs = slice(kv * KT, (kv + 1) * KT)
                    qs = slice(t * QT + v0, (t + 1) * QT)
                    nc.tensor.matmul(s2[:, v0:QT], kT_sb[0:64, g, kvs],
                                     qT_sb[0:64, g, qs], start=True, stop=True)
                    nc.tensor.matmul(s2[:, QT + v0:], kT_sb[64:128, g, kvs],
                                     qT_sb[64:128, g, qs], start=True, stop=True)
                    p2 = pwork.tile([P, 2 * QT], F16, tag="p", name="p2")
                    if r >= 1:
                        s2r = s2[:, :].rearrange("p (i q) -> p i q", i=2)
                        p2r = p2[:, :].rearrange("p (i q) -> p i q", i=2)
                        nc.scalar.activation(p2r[:, :, v0:], s2r[:, :, v0:], EXP)
                        nc.vector.tensor_mul(p2r[:, :, v0:], p2r[:, :, v0:],
                                             msk_sb[:, :, 0:QT - v0])
                    else:
                        nc.scalar.activation(p2, s2, EXP)
                        if r == 0:
                            nc.vector.tensor_mul(
                                p2[:, :].rearrange("p (i q) -> p i q", i=2),
                                p2[:, :].rearrange("p (i q) -> p i q", i=2),
                                msk_sb)
                    if pend is not None:
                        pr = kv - 1 - 4 * t
                        mid = (MID_D[pr] if pr >= 0 else MID_N) if t > 0 else 0
                        flush_pv(pend, g, av_a, av_b, n_kv, mid)
                    pend = (kv, p2, v0)
                if t > 0:
                    q.pull(SEG_END)
                flush_pv(pend, g, av_a, av_b, n_kv, 0)
                # normalize: aT = av / den, denominators shifted across
                # partition halves via a small SBUF->SBUF DMA
                rca = misc.tile([P, QT], F32, tag="m", name="rca")
                rcb = misc.tile([P, QT], F32, tag="m", name="rcb")
                rc2 = misc.tile([P, QT], F32, tag="m", name="rc2")
                nc.vector.reciprocal_approx_fast(rca, av_a)
                nc.sync.dma_start(out=rc2[0:64, :], in_=rca[64:128, :])
                nc.vector.reciprocal_approx_fast(rcb, av_b)
                nc.sync.dma_start(out=rc2[64:128, :], in_=rcb[0:64, :])
                nc.vector.tensor_mul(aT_sb[0:64, g, t, :], av_a[0:64, :],
                                     rc2[0:64, :])
                nc.vector.tensor_mul(aT_sb[64:128, g, t, :], av_b[64:128, :],
                                     rc2[64:128, :])

            for t in range(NQT):
                for g in range(2):
                    if t == 3 and g == 1:
                        q.reserve = 0
                    attention(t, g)
            q.drain()

            # ---- tail: o-projection of q-tile 2 fills the final normalize
            # window, then the last q tile with copies split across DVE/ACT ----
            for mt in range(D // P):
                emit_o(2, mt, "scalar" if mt % 2 else "vector")
            for mt in range(D // P):
                emit_o(3, mt, "scalar" if mt % 2 else "vector")

    nc.compile()
    _NC_CACHE["nc"] = nc
    return nc


def _make_strip():
    # strip[p, u] = 1.0 iff p <= u; diagonal kv offset r uses cols [0, 512-128r)
    pp = np.arange(P)[:, None]
    uu = np.arange(QT)[None, :]
    return (pp <= uu).astype(np.float16)


def _in_maps(x, Wq, bq, Wk, bk, Wv, bv, Wo):
    scale = np.float32(1.0 / np.sqrt(HD))
    strip = _make_strip()
    maps = []
    for core in range(NCORES):
        b, g = divmod(core, GROUPS)
        csl = slice(g * CD, (g + 1) * CD)
        maps.append({
            "xt": np.ascontiguousarray(x[b].T).astype(np.float16),
            "wq": np.ascontiguousarray(Wq[:, csl] * scale).astype(np.float16),
            "wk": np.ascontiguousarray(Wk[:, csl]).astype(np.float16),
            "wv": np.ascontiguousarray(Wv[:, csl]).astype(np.float16),
            "wo": np.ascontiguousarray(Wo[csl, :]).astype(np.float16),
            "bq": np.ascontiguousarray(bq[csl] * scale).astype(np.float32),
            "bk": np.ascontiguousarray(bk[csl]).astype(np.float32),
            "bv": np.ascontiguousarray(bv[csl]).reshape(HPG, HD).astype(np.float32),
            "msk": strip,
        })
    return maps


def kernel_with_results(x, Wq, bq, Wk, bk, Wv, bv, Wo, bo, trace=False):
    nc = _build_nc()
    maps = _in_maps(x, Wq, bq, Wk, bk, Wv, bv, Wo)
    kwargs = {}
    if trace:
        kwargs = dict(trace=True, trace_cores=[0])
    res = run_bass_kernel_spmd(nc, maps, core_ids=list(range(NCORES)), **kwargs)
    out = np.zeros((B, S, D), dtype=np.float32)
    for b in range(B):
        acc = np.zeros((D, S), dtype=np.float32)
        for g in range(GROUPS):
            acc += res.results[b * GROUPS + g]["ot"]
        out[b] = acc.T + np.asarray(bo, dtype=np.float32)[None, :]
    return out, res


def kernel(x, Wq, bq, Wk, bk, Wv, bv, Wo, bo):
    out, _ = kernel_with_results(x, Wq, bq, Wk, bk, Wv, bv, Wo, bo, trace=False)
    return out
